# revision 2
# baseline (speedup 1.0000x reference)
"""ConvFace GNN kernel v3 — on-device gather, fp16 I/O to minimize tunnel bytes.

Per core c: batch b=c//2, face half h=c%2 (5000 faces, T=40 tiles of 128).
Device gets fea[b] transposed as an fp16 [20000, 64] DRAM table and gathers
cat rows with per-partition indirect DMAs (17 per face tile).  Math per tile:
  g    = A^T xs_aug   (A = [Wk^T Wq; bk^T Wq]/sqrt(128), xs augmented by 1)
  att  = softmax_k(g . cat_k)
  agg  = sum_k att_k cat_k
  y    = Wc agg          (bc dropped; cancels in BN)
BatchNorm statistics + affine + ReLU run on the host from fp16 y.
"""

import numpy as np

import concourse.bass as bass
import concourse.bacc as bacc
import concourse.mybir as mybir
import concourse.tile as tile

AF = mybir.ActivationFunctionType
ALU = mybir.AluOpType
F32 = mybir.dt.float32
F16 = mybir.dt.float16
I16 = mybir.dt.int16
I32 = mybir.dt.int32

B, C, F, FP, K, O = 4, 64, 20000, 10000, 16, 128
K1 = K + 1
NCORES = 8
SQRT_DK = float(np.sqrt(128.0))
BN_EPS = 1e-5

_T = 40          # face tiles of 128 per core
_FPC = FP // 2   # valid faces per core


def build_nc_v4(T=_T, fpc_valid=_FPC, num_devices=NCORES):
    nc = bacc.Bacc(trn_type="TRN2", num_devices=num_devices)
    fea_half = nc.dram_tensor("fea_half", [F // 2, C], F16, kind="ExternalInput")
    idx16t = nc.dram_tensor("idx16", [128, T * K1], I16, kind="ExternalInput")
    a_aug = nc.dram_tensor("a_aug", [C + 1, C], F16, kind="ExternalInput")
    wct = nc.dram_tensor("wct", [C, O], F16, kind="ExternalInput")
    ident = nc.dram_tensor("ident", [128, 128], F16, kind="ExternalInput")
    y16 = nc.dram_tensor("y16", [128, fpc_valid], F16, kind="ExternalOutput")
    cc_in = nc.dram_tensor("cc_in", [F // 2, C], F16, kind="Internal")
    fea16 = nc.dram_tensor("fea16", [F, C], F16, kind="Internal")

    with tile.TileContext(nc) as tc:
        with (
            tc.tile_pool(name="singles", bufs=1) as singles,
            tc.tile_pool(name="g16", bufs=3) as g16_pool,
            tc.tile_pool(name="cat", bufs=2) as cat_pool,
            tc.tile_pool(name="mid", bufs=2) as mid,
            tc.tile_pool(name="prod", bufs=2) as prod_pool,
            tc.tile_pool(name="prod2", bufs=2) as prod2_pool,
            tc.tile_pool(name="small", bufs=3) as small,
            tc.tile_pool(name="pst", bufs=2, space="PSUM") as pst,
            tc.tile_pool(name="pgf", bufs=2, space="PSUM") as pgf,
            tc.tile_pool(name="py", bufs=2, space="PSUM") as py,
        ):
            # pair AllGather reconstructs the full per-batch gather table:
            # core 2b holds rows [0, F/2), core 2b+1 rows [F/2, F)
            nc.sync.dma_start(out=cc_in[:], in_=fea_half[:])
            nc.gpsimd.collective_compute(
                "AllGather",
                ALU.bypass,
                replica_groups=[[2 * b, 2 * b + 1] for b in range(num_devices // 2)],
                ins=[cc_in[:]],
                outs=[fea16[:]],
            )

            idx_sb16 = singles.tile([128, T * K1], I16)
            nc.sync.dma_start(out=idx_sb16[:], in_=idx16t[:])
            idx_sb = singles.tile([128, T * K1], I32)
            nc.vector.tensor_copy(out=idx_sb[:], in_=idx_sb16[:])
            a_sb = singles.tile([C + 1, C], F16)
            nc.sync.dma_start(out=a_sb[:], in_=a_aug[:])
            wct_sb = singles.tile([C, O], F16)
            nc.sync.dma_start(out=wct_sb[:], in_=wct[:])
            ident_sb = singles.tile([128, 128], F16)
            nc.sync.dma_start(out=ident_sb[:], in_=ident[:])

            xsT_aug = singles.tile([C + 1, 128], F16)
            nc.vector.memset(xsT_aug[C : C + 1, :], 1.0)
            zero_t = singles.tile([128, 1], F32)
            nc.vector.memset(zero_t[:], 0.0)
            ybuf = singles.tile([128, T * 128], F16)

            for t in range(T):
                cat16 = g16_pool.tile([128, K1 * C], F16)
                for j in range(K1):
                    nc.gpsimd.indirect_dma_start(
                        out=cat16[:, j * C : (j + 1) * C],
                        out_offset=None,
                        in_=fea16[:],
                        in_offset=bass.IndirectOffsetOnAxis(
                            ap=idx_sb[:, t * K1 + j : t * K1 + j + 1], axis=0
                        ),
                    )
                cat = cat_pool.tile([128, K1, C], F32)
                nc.vector.tensor_copy(
                    out=cat[:].rearrange("p k c -> p (k c)"), in_=cat16[:]
                )

                # ---- g (face-major): xsT via PE, then (xs_aug)^T @ A ----
                xsT_psum = pst.tile([C, 128], F16, tag="pst")
                nc.tensor.transpose(xsT_psum[:], cat16[:, 0:C], ident_sb[:])
                nc.scalar.activation(xsT_aug[0:C, :], xsT_psum[:], AF.Copy)
                gf_psum = pgf.tile([128, C], F32)
                nc.tensor.matmul(
                    gf_psum[:], lhsT=xsT_aug[:], rhs=a_sb[:], start=True, stop=True
                )
                gf_sb = mid.tile([128, C], F32, tag="gf")
                nc.scalar.activation(gf_sb[:], gf_psum[:], AF.Copy)

                # ---- logits[f,k] = sum_c gf[f,c] * cat[f,k,c] ----
                prod = prod_pool.tile([128, K1, C], F32)
                gf_b = gf_sb[:].unsqueeze(1).to_broadcast([128, K1, C])
                nc.vector.tensor_tensor(out=prod[:], in0=cat[:], in1=gf_b, op=ALU.mult)
                logits = small.tile([128, K1], F32, tag="logits")
                nc.vector.tensor_reduce(
                    out=logits[:], in_=prod[:], axis=mybir.AxisListType.X, op=ALU.add
                )

                # ---- softmax over k (logits small; skip max-sub) ----
                attu = small.tile([128, K1], F32, tag="attu")
                ssum = small.tile([128, 1], F32, tag="ssum")
                nc.scalar.activation(
                    attu[:], logits[:], AF.Exp, bias=zero_t[:], accum_out=ssum[:]
                )
                rinv = small.tile([128, 1], F32, tag="rinv")
                nc.vector.reciprocal(rinv[:], ssum[:])
                att = small.tile([128, K1], F32, tag="att")
                nc.vector.tensor_scalar(
                    out=att[:], in0=attu[:], scalar1=rinv[:], scalar2=None, op0=ALU.mult
                )

                # ---- agg[f,c] = sum_k att[f,k] * cat[f,k,c] ----
                prod2 = prod2_pool.tile([128, K1, C], F32)
                att_b = att[:].unsqueeze(2).to_broadcast([128, K1, C])
                nc.vector.tensor_tensor(
                    out=prod2[:], in0=cat[:], in1=att_b, op=ALU.mult
                )
                agg = mid.tile([128, C], F32, tag="agg")
                nc.vector.tensor_reduce(
                    out=agg[:], in_=prod2[:].rearrange("p k c -> p c k"),
                    axis=mybir.AxisListType.X, op=ALU.add,
                )
                agg16 = mid.tile([128, C], F16, tag="agg16")
                nc.scalar.activation(agg16[:], agg[:], AF.Copy)

                # ---- y = Wc @ agg ----
                aggT_psum = pst.tile([C, 128], F16, tag="pst")
                nc.tensor.transpose(aggT_psum[:], agg16[:], ident_sb[:])
                aggT16 = mid.tile([C, 128], F16, tag="aggT")
                nc.scalar.activation(aggT16[:], aggT_psum[:], AF.Copy)
                y_psum = py.tile([O, 128], F32)
                nc.tensor.matmul(
                    y_psum[:], lhsT=wct_sb[:], rhs=aggT16[:], start=True, stop=True
                )
                nc.scalar.activation(
                    ybuf[:, t * 128 : (t + 1) * 128], y_psum[:], AF.Copy
                )

            nc.sync.dma_start(out=y16[:], in_=ybuf[:, 0:fpc_valid])
    nc.compile()
    return nc


def prep_idx16(cat_idx, T):
    """cat_idx [n, K1] -> int16 [128, T*K1] with idx[p, t*K1+k] = cat_idx[t*128+p, k]."""
    fpp = T * 128
    pad = fpp - cat_idx.shape[0]
    ci = (
        np.concatenate([cat_idx, np.zeros((pad, K1), cat_idx.dtype)], 0)
        if pad
        else cat_idx
    )
    return np.ascontiguousarray(
        ci.reshape(T, 128, K1).transpose(1, 0, 2).reshape(128, T * K1)
    ).astype(np.int16)


def prep_weights16(Wk, bk, Wq, bq, Wc):
    Wk = np.asarray(Wk, np.float64)
    Wq = np.asarray(Wq, np.float64)
    bk = np.asarray(bk, np.float64)
    a_mat = (Wk.T @ Wq) / SQRT_DK                  # [c, j]
    u = (Wq.T @ bk) / SQRT_DK                      # [j]
    a_aug = np.concatenate([a_mat, u[None, :]], 0).astype(np.float16)  # [C+1, C]
    wct = np.ascontiguousarray(np.asarray(Wc, np.float32).T).astype(np.float16)
    ident = np.eye(128, dtype=np.float16)
    return a_aug, wct, ident


def prepare_in_maps(fea, ring_n, pool_idx, Wk, bk, Wq, bq, Wc):
    fea = np.asarray(fea, np.float32)
    ring_n = np.asarray(ring_n)
    pool_idx = np.asarray(pool_idx)
    T, fpc = _T, _FPC
    a_aug, wct, ident = prep_weights16(Wk, bk, Wq, bq, Wc)
    fea16 = [
        np.ascontiguousarray(fea[b].T).astype(np.float16) for b in range(B)
    ]  # [F, C] per batch
    in_maps = []
    for c in range(NCORES):
        b, h = c // 2, c % 2
        ci = np.concatenate(
            [pool_idx[h * fpc : (h + 1) * fpc, None],
             ring_n[b, h * fpc : (h + 1) * fpc]], axis=1,
        )
        in_maps.append(
            {
                "fea_half": fea16[b][h * (F // 2) : (h + 1) * (F // 2)],
                "idx16": prep_idx16(ci, T),
                "a_aug": a_aug,
                "wct": wct,
                "ident": ident,
            }
        )
    return in_maps


def postprocess(y16_list, gamma, beta):
    """y16_list: per-core [128, FPC] fp16 -> full [B, O, FP] f32 after BN+ReLU."""
    fpc = _FPC
    y = np.empty((B, O, FP), np.float32)
    for c in range(NCORES):
        b, h = c // 2, c % 2
        y[b, :, h * fpc : (h + 1) * fpc] = y16_list[c][:, 0:fpc].astype(np.float32)
    mean = y.mean(axis=(0, 2), keepdims=True)
    var = y.var(axis=(0, 2), keepdims=True)
    yn = (y - mean) / np.sqrt(var + BN_EPS)
    yn = yn * np.asarray(gamma, np.float32)[None, :, None] + np.asarray(
        beta, np.float32
    )[None, :, None]
    return np.maximum(yn, 0.0)


_NC = None


def kernel(fea, ring_n, pool_idx, pos_embed=None, Wk=None, bk=None, Wq=None,
           bq=None, Wc=None, bc=None, gamma=None, beta=None):
    from concourse.bass_utils import run_bass_kernel_spmd

    in_maps = prepare_in_maps(fea, ring_n, pool_idx, Wk, bk, Wq, bq, Wc)
    global _NC
    if _NC is None:
        _NC = build_nc_v4(T=_T, fpc_valid=_FPC, num_devices=NCORES)
    res = run_bass_kernel_spmd(_NC, in_maps, core_ids=list(range(NCORES)))
    return postprocess([res.results[c]["y16"] for c in range(NCORES)], gamma, beta)
